# revision 25
# baseline (speedup 1.0000x reference)
"""Trainium2 Bass kernel for ContextQueryAttention (trilinear attention).

Math (per batch b; C:[D,N], Q:[D,M], W0:[3D], b0:[1]):
    Ct = C.T, Qt = Q.T
    S[n,m] = Ct@w_c [n] + Qt@w_q [m] + sum_d Ct[n,d]*w_qc[d]*Qt[m,d] + b0
    S_row = softmax_m(S), S_col = softmax_n(S)
    A  = S_row @ Qt                       # (N, D)
    Bt = (S_row @ S_col.T) @ Ct           # (N, D)

Key algebraic restructurings used here:
  * Bt = S_row @ (S_col.T @ Ct)  -- drops the N x N intermediate entirely
    (805 MFLOP/batch -> 134 MFLOP/batch).
  * softmax_m is invariant to per-row constants, softmax_n to per-column
    constants, so the row path only needs the q-score bias and the col path
    only the c-score bias; b0 cancels everywhere.
  * Input magnitudes are O(5), so exp() needs no max-subtraction.
  * Softmax denominators come for free as extra all-ones columns fused
    into the consuming matmuls; normalization folds into per-partition
    scalar multiplies after the matmuls.
  * All matmuls run in float32r (full-rate fp32); moving free sizes kept
    even (hw requirement) by duplicating the fused score/ones columns.

Distribution: ALL 64 batches on ONE core, looped with a hardware For_i
(unroll=4, staggered_reset back-edges, merged 3D output DMAs).
Rationale (measured on the axon-tunneled PJRT path):
  * Per-call dispatch overhead dominates the amortized exec time and
    scales with the number of devices (8-core ~12ms, 2-core ~8ms,
    1-core ~2.4-3ms per call) while the device compute (<1ms) hides
    behind the dispatch pipeline. One core minimizes the metric; the
    sharding_hint's 8-way data parallelism loses 4-5x here.
  * Steady-state per-call cost also grows with NEFF size (instruction
    stream + DMA descriptors): fully unrolling 64 batches costs ~1.5ms
    extra per call. For_i with a 4-batch unrolled body keeps the stream
    ~700 instructions; staggered_reset avoids the ~2us full-barrier
    back-edge and restores cross-batch engine overlap; one 3D DMA per
    output per batch replaces 16 chunk DMAs.
"""

import numpy as np

import concourse.bass as bass
import concourse.bacc as bacc
import concourse.tile as tile
from concourse import mybir
from concourse.bass import ts
from concourse.bass_utils import run_bass_kernel_spmd
from concourse.masks import make_identity

F32 = mybir.dt.float32
F32R = mybir.dt.float32r
BF16 = mybir.dt.bfloat16

# Problem shape (hardcoded per spec)
B, D, N, M = 64, 128, 1024, 256
NCORES = 1
BPC = B // NCORES  # batches per core
NK = N // 128      # context chunks (8)
MJ = M // 128      # query chunks (2)


def build_kernel(
    bpc: int = BPC,
    dynamic: bool = True,
    unroll: int = 4,
    merged_out: bool = True,
    staggered: bool = True,
    out_bf16: bool = False,
    col_bf16: bool = True,
) -> bass.Bass:
    ODT = BF16 if out_bf16 else F32
    # col-path operand dtype: bf16 lifts the fp32r small-moving-dim (130 < 256)
    # 4-cycles/row matmul penalty to full rate, with no extra conversions
    # (e_col is an ACT output, ct a PSUM->SBUF copy; both cast on write).
    CDT = BF16 if col_bf16 else F32R
    nc = bacc.Bacc("TRN2", target_bir_lowering=False, debug=False)

    C8 = nc.dram_tensor("C", [bpc, D, N], F32, kind="ExternalInput").ap()
    Q8 = nc.dram_tensor("Q", [bpc, D, M], F32, kind="ExternalInput").ap()
    W0 = nc.dram_tensor("W0", [3 * D], F32, kind="ExternalInput").ap()
    A8 = nc.dram_tensor("A", [bpc, N, D], ODT, kind="ExternalOutput").ap()
    B8 = nc.dram_tensor("Bt", [bpc, N, D], ODT, kind="ExternalOutput").ap()

    # flat row views for dynamic (runtime-index) batch addressing
    Cf = C8.rearrange("b d n -> (b d) n")
    Qf = Q8.rearrange("b d n -> (b d) n")
    Af = A8.rearrange("b n d -> (b n) d")
    Bf = B8.rearrange("b n d -> (b n) d")

    with tile.TileContext(nc) as tc:
        with (
            tc.tile_pool(name="singles", bufs=1) as singles,
            tc.tile_pool(name="inp", bufs=2) as pool_in,
            tc.tile_pool(name="scaled", bufs=2) as pool_sc,
            tc.tile_pool(name="ct", bufs=2) as pool_ct,
            tc.tile_pool(name="e", bufs=2) as pool_e,
            tc.tile_pool(name="qtg", bufs=2) as pool_qtg,
            tc.tile_pool(name="small", bufs=2) as pool_sm,
            tc.tile_pool(name="out", bufs=2) as pool_out,
            tc.tile_pool(name="pp_t", bufs=2, space="PSUM") as pp_t,
            tc.tile_pool(name="pp_x", bufs=2, space="PSUM") as pp_x,
            tc.tile_pool(name="pp_xt", bufs=2, space="PSUM") as pp_xt,
        ):
            # --- constants ---
            # wvec cols: w_q, w_q, w_c, w_c, w_qc  (score columns doubled so
            # fused matmul moving sizes stay even, as float32r requires)
            wvec = singles.tile([D, 5], F32)
            for i, s in enumerate((0, 0, 1, 1, 2)):
                nc.sync.dma_start(
                    out=wvec[:, i : i + 1],
                    in_=W0[s * D : (s + 1) * D].rearrange("(p o) -> p o", o=1),
                )
            w_qc = wvec[:, 4:5]
            ones2 = singles.tile([128, 2], F32)
            nc.vector.memset(ones2, 1.0)
            ident_f32 = singles.tile([128, 128], F32)
            make_identity(nc, ident_f32)
            ident = singles.tile([128, 128], F32R)
            nc.vector.tensor_copy(out=ident, in_=ident_f32)

            import contextlib

            loop_ctx = (
                tc.For_i(0, bpc, unroll, staggered_reset=staggered)
                if dynamic
                else contextlib.nullcontext(0)
            )
            with loop_ctx as bdyn:
              for bstat in range(unroll if dynamic else bpc):
                b = (bdyn + bstat) if dynamic else bstat
                u = bstat % 2  # alternate tags so consecutive batches overlap
                cb = pool_in.tile([D, N], F32R, tag=f"cb{u}")
                qb = pool_in.tile([D, M], F32R, tag=f"qb{u}")
                nc.sync.dma_start(out=cb, in_=Cf[ts(b, D), :].bitcast(F32R))
                nc.sync.dma_start(out=qb, in_=Qf[ts(b, D), :].bitcast(F32R))

                # scaled inputs with fused (doubled) score columns
                # cswq = [C * w_qc | w_q w_q]  -> rhs for X^T and QS matmuls
                cswq = pool_sc.tile([D, N + 2], F32R, tag=f"cswq{u}")
                nc.vector.tensor_scalar_mul(out=cswq[:, 0:N], in0=cb, scalar1=w_qc)
                nc.vector.tensor_copy(out=cswq[:, N : N + 2], in_=wvec[:, 0:2])
                # qswc = [Q * w_qc | w_c w_c]  -> rhs for X matmuls
                qswc = pool_sc.tile([D, M + 2], F32R, tag=f"qswc{u}")
                nc.vector.tensor_scalar_mul(out=qswc[:, 0:M], in0=qb, scalar1=w_qc)
                nc.vector.tensor_copy(out=qswc[:, M : M + 2], in_=wvec[:, 2:4])

                # --- transposes: ct_k = [Ct_k | 1 1], qtg_j = [Qt_j | 1 1 | G_j]
                ct = pool_ct.tile([128, NK, D + 2], CDT, tag=f"ct{u}")
                for k in range(NK):
                    pt = pp_t.tile([128, 128], F32R, tag="pt")
                    nc.tensor.transpose(pt, cb[:, k * 128 : (k + 1) * 128], ident)
                    nc.vector.tensor_copy(out=ct[:, k, 0:D], in_=pt.bitcast(F32))
                    nc.vector.tensor_copy(out=ct[:, k, D : D + 2], in_=ones2)

                qtg = pool_qtg.tile([128, MJ, 2 * D + 2], F32R, tag=f"qtg{u}")
                for j in range(MJ):
                    pt = pp_t.tile([128, 128], F32R, tag="pt")
                    nc.tensor.transpose(pt, qb[:, j * 128 : (j + 1) * 128], ident)
                    nc.vector.tensor_copy(out=qtg[:, j, 0:D], in_=pt)
                    nc.vector.tensor_copy(out=qtg[:, j, D : D + 2], in_=ones2)

                # --- X [n,m] chunks + col-softmax numerator E ---
                e_col = pool_e.tile([128, NK, M], CDT, tag=f"e_col{u}")
                for k in range(NK):
                    px = pp_x.tile([128, M + 2], F32, tag="px")
                    nc.tensor.matmul(
                        px, cb[:, k * 128 : (k + 1) * 128], qswc, start=True, stop=True
                    )
                    cs_k = pool_sm.tile([128, 1], F32, tag=f"cs{k}_{u}")
                    nc.vector.tensor_copy(out=cs_k, in_=px[:, M : M + 1])
                    nc.scalar.activation(
                        out=e_col[:, k, :],
                        in_=px[:, 0:M],
                        func=mybir.ActivationFunctionType.Exp,
                        bias=cs_k,
                    )

                # --- X^T [m,n] chunks + row-softmax numerator E' ---
                e_row = pool_e.tile([128, MJ, N], F32R, tag=f"e_row{u}")
                for j in range(MJ):
                    qbj = qb[:, j * 128 : (j + 1) * 128]
                    pxt = pp_xt.tile([128, N], F32, tag="pxt")
                    for h in range(N // 512):
                        nc.tensor.matmul(
                            pxt[:, h * 512 : (h + 1) * 512],
                            qbj,
                            cswq[:, h * 512 : (h + 1) * 512],
                            start=True,
                            stop=True,
                        )
                    pq = pp_t.tile([128, 128], F32, tag="pt")
                    nc.tensor.matmul(
                        pq[:, 0:2], qbj, cswq[:, N : N + 2], start=True, stop=True
                    )
                    qs_j = pool_sm.tile([128, 1], F32, tag=f"qs{j}_{u}")
                    nc.vector.tensor_copy(out=qs_j, in_=pq[:, 0:1])
                    nc.scalar.activation(
                        out=e_row[:, j, :],
                        in_=pxt,
                        func=mybir.ActivationFunctionType.Exp,
                        bias=qs_j,
                    )

                # --- col path: G_j = normalize(E^T @ [Ct|1 1]) ---
                for j in range(MJ):
                    pg = pp_t.tile([128, D + 2], F32, tag="pt")
                    for k in range(NK):
                        nc.tensor.matmul(
                            pg,
                            e_col[:, k, j * 128 : (j + 1) * 128],
                            ct[:, k, :],
                            start=(k == 0),
                            stop=(k == NK - 1),
                        )
                    rcol = pool_sm.tile([128, 1], F32, tag=f"rcol{j}_{u}")
                    nc.vector.reciprocal(out=rcol, in_=pg[:, D : D + 1])
                    nc.vector.tensor_scalar_mul(
                        out=qtg[:, j, D + 2 : 2 * D + 2], in0=pg[:, 0:D], scalar1=rcol
                    )

                # --- row path: [A | rowsum rowsum | Bt] = E'^T @ [Qt|1 1|G] ---
                if merged_out:
                    oabs = pool_out.tile([128, NK, 2 * D], ODT, tag=f"oabs{u}")
                for k in range(NK):
                    pab = pp_x.tile([128, 2 * D + 2], F32, tag="px")
                    for j in range(MJ):
                        nc.tensor.matmul(
                            pab,
                            e_row[:, j, k * 128 : (k + 1) * 128],
                            qtg[:, j, :],
                            start=(j == 0),
                            stop=(j == MJ - 1),
                        )
                    rrow = pool_sm.tile([128, 1], F32, tag=f"rrow{k}_{u}")
                    nc.vector.reciprocal(out=rrow, in_=pab[:, D : D + 1])
                    oab = oabs[:, k, :] if merged_out else pool_out.tile(
                        [128, 2 * D], ODT, tag="oab"
                    )
                    nc.vector.tensor_scalar_mul(
                        out=oab[:, 0:D], in0=pab[:, 0:D], scalar1=rrow
                    )
                    nc.vector.tensor_scalar_mul(
                        out=oab[:, D : 2 * D], in0=pab[:, D + 2 : 2 * D + 2], scalar1=rrow
                    )
                    if not merged_out:
                        nc.sync.dma_start(
                            out=Af[ts(b * NK + k, 128), :], in_=oab[:, 0:D]
                        )
                        nc.sync.dma_start(
                            out=Bf[ts(b * NK + k, 128), :], in_=oab[:, D : 2 * D]
                        )
                if merged_out:
                    # one 3D DMA per output per batch: SBUF [p, k, d] -> DRAM
                    # rows (b*N + k*128 + p)
                    nc.sync.dma_start(
                        out=Af[ts(b, N), :].rearrange("(k p) d -> p k d", p=128),
                        in_=oabs[:, :, 0:D],
                    )
                    nc.sync.dma_start(
                        out=Bf[ts(b, N), :].rearrange("(k p) d -> p k d", p=128),
                        in_=oabs[:, :, D : 2 * D],
                    )
    nc.finalize()
    return nc


_NC_CACHE = None


def kernel(C, Q, W0, b0, _trace=False):
    global _NC_CACHE
    if _NC_CACHE is None:
        _NC_CACHE = build_kernel()
    nc = _NC_CACHE

    C = np.ascontiguousarray(np.asarray(C, dtype=np.float32))
    Q = np.ascontiguousarray(np.asarray(Q, dtype=np.float32))
    W0 = np.ascontiguousarray(np.asarray(W0, dtype=np.float32))

    in_maps = [
        {
            "C": C[i * BPC : (i + 1) * BPC],
            "Q": Q[i * BPC : (i + 1) * BPC],
            "W0": W0,
        }
        for i in range(NCORES)
    ]
    res = run_bass_kernel_spmd(nc, in_maps, core_ids=list(range(NCORES)))
    A = np.concatenate(
        [np.asarray(res.results[i]["A"]) for i in range(NCORES)], axis=0
    ).astype(np.float32)
    Bt = np.concatenate(
        [np.asarray(res.results[i]["Bt"]) for i in range(NCORES)], axis=0
    ).astype(np.float32)
    return (A, Bt)


# revision 26
# speedup vs baseline: 1.0889x; 1.0889x over previous
"""Trainium2 Bass kernel for ContextQueryAttention (trilinear attention).

Math (per batch b; C:[D,N], Q:[D,M], W0:[3D], b0:[1]):
    Ct = C.T, Qt = Q.T
    S[n,m] = Ct@w_c [n] + Qt@w_q [m] + sum_d Ct[n,d]*w_qc[d]*Qt[m,d] + b0
    S_row = softmax_m(S), S_col = softmax_n(S)
    A  = S_row @ Qt                       # (N, D)
    Bt = (S_row @ S_col.T) @ Ct           # (N, D)

Key algebraic restructurings used here:
  * Bt = S_row @ (S_col.T @ Ct)  -- drops the N x N intermediate entirely
    (805 MFLOP/batch -> 134 MFLOP/batch).
  * softmax_m is invariant to per-row constants, softmax_n to per-column
    constants, so the row path only needs the q-score bias and the col path
    only the c-score bias; b0 cancels everywhere.
  * Input magnitudes are O(5), so exp() needs no max-subtraction.
  * Softmax denominators come for free as extra all-ones columns fused
    into the consuming matmuls; normalization folds into per-partition
    scalar multiplies after the matmuls.
  * All matmuls run in float32r (full-rate fp32); moving free sizes kept
    even (hw requirement) by duplicating the fused score/ones columns.

Distribution: ALL 64 batches on ONE core, looped with a hardware For_i
(unroll=4, staggered_reset back-edges, merged 3D output DMAs).
Rationale (measured on the axon-tunneled PJRT path):
  * Per-call dispatch overhead dominates the amortized exec time and
    scales with the number of devices (8-core ~12ms, 2-core ~8ms,
    1-core ~2.4-3ms per call) while the device compute (<1ms) hides
    behind the dispatch pipeline. One core minimizes the metric; the
    sharding_hint's 8-way data parallelism loses 4-5x here.
  * Steady-state per-call cost also grows with NEFF size (instruction
    stream + DMA descriptors): fully unrolling 64 batches costs ~1.5ms
    extra per call. For_i with a 4-batch unrolled body keeps the stream
    ~700 instructions; staggered_reset avoids the ~2us full-barrier
    back-edge and restores cross-batch engine overlap; one 3D DMA per
    output per batch replaces 16 chunk DMAs.
"""

import numpy as np

import concourse.bass as bass
import concourse.bacc as bacc
import concourse.tile as tile
from concourse import mybir
from concourse.bass import ts
from concourse.bass_utils import run_bass_kernel_spmd
from concourse.masks import make_identity

F32 = mybir.dt.float32
F32R = mybir.dt.float32r
BF16 = mybir.dt.bfloat16

# Problem shape (hardcoded per spec)
B, D, N, M = 64, 128, 1024, 256
NCORES = 1
BPC = B // NCORES  # batches per core
NK = N // 128      # context chunks (8)
MJ = M // 128      # query chunks (2)


def build_kernel(
    bpc: int = BPC,
    dynamic: bool = True,
    unroll: int = 4,
    merged_out: bool = True,
    staggered: bool = True,
    out_bf16: bool = False,
    col_bf16: bool = False,
) -> bass.Bass:
    ODT = BF16 if out_bf16 else F32
    # col-path operand dtype: bf16 lifts the fp32r small-moving-dim (130 < 256)
    # 4-cycles/row matmul penalty to full rate, with no extra conversions
    # (e_col is an ACT output, ct a PSUM->SBUF copy; both cast on write).
    CDT = BF16 if col_bf16 else F32R
    nc = bacc.Bacc("TRN2", target_bir_lowering=False, debug=False)

    C8 = nc.dram_tensor("C", [bpc, D, N], F32, kind="ExternalInput").ap()
    Q8 = nc.dram_tensor("Q", [bpc, D, M], F32, kind="ExternalInput").ap()
    W0 = nc.dram_tensor("W0", [3 * D], F32, kind="ExternalInput").ap()
    A8 = nc.dram_tensor("A", [bpc, N, D], ODT, kind="ExternalOutput").ap()
    B8 = nc.dram_tensor("Bt", [bpc, N, D], ODT, kind="ExternalOutput").ap()

    # flat row views for dynamic (runtime-index) batch addressing
    Cf = C8.rearrange("b d n -> (b d) n")
    Qf = Q8.rearrange("b d n -> (b d) n")
    Af = A8.rearrange("b n d -> (b n) d")
    Bf = B8.rearrange("b n d -> (b n) d")

    with tile.TileContext(nc) as tc:
        with (
            tc.tile_pool(name="singles", bufs=1) as singles,
            tc.tile_pool(name="inp", bufs=2) as pool_in,
            tc.tile_pool(name="scaled", bufs=2) as pool_sc,
            tc.tile_pool(name="ct", bufs=2) as pool_ct,
            tc.tile_pool(name="e", bufs=2) as pool_e,
            tc.tile_pool(name="qtg", bufs=2) as pool_qtg,
            tc.tile_pool(name="small", bufs=2) as pool_sm,
            tc.tile_pool(name="out", bufs=2) as pool_out,
            tc.tile_pool(name="pp_t", bufs=2, space="PSUM") as pp_t,
            tc.tile_pool(name="pp_x", bufs=2, space="PSUM") as pp_x,
            tc.tile_pool(name="pp_xt", bufs=2, space="PSUM") as pp_xt,
        ):
            # --- constants ---
            # wvec cols: w_q, w_q, w_c, w_c, w_qc  (score columns doubled so
            # fused matmul moving sizes stay even, as float32r requires)
            wvec = singles.tile([D, 5], F32)
            for i, s in enumerate((0, 0, 1, 1, 2)):
                nc.sync.dma_start(
                    out=wvec[:, i : i + 1],
                    in_=W0[s * D : (s + 1) * D].rearrange("(p o) -> p o", o=1),
                )
            w_qc = wvec[:, 4:5]
            ones2 = singles.tile([128, 2], F32)
            nc.vector.memset(ones2, 1.0)
            ident_f32 = singles.tile([128, 128], F32)
            make_identity(nc, ident_f32)
            ident = singles.tile([128, 128], F32R)
            nc.vector.tensor_copy(out=ident, in_=ident_f32)

            import contextlib

            loop_ctx = (
                tc.For_i(0, bpc, unroll, staggered_reset=staggered)
                if dynamic
                else contextlib.nullcontext(0)
            )
            with loop_ctx as bdyn:
              for bstat in range(unroll if dynamic else bpc):
                b = (bdyn + bstat) if dynamic else bstat
                u = bstat % 2  # alternate tags so consecutive batches overlap
                cb = pool_in.tile([D, N], F32R, tag=f"cb{u}")
                qb = pool_in.tile([D, M], F32R, tag=f"qb{u}")
                nc.sync.dma_start(out=cb, in_=Cf[ts(b, D), :].bitcast(F32R))
                nc.sync.dma_start(out=qb, in_=Qf[ts(b, D), :].bitcast(F32R))

                # scaled inputs with fused (doubled) score columns
                # cswq = [C * w_qc | w_q w_q]  -> rhs for X^T and QS matmuls
                cswq = pool_sc.tile([D, N + 2], F32R, tag=f"cswq{u}")
                nc.vector.tensor_scalar_mul(out=cswq[:, 0:N], in0=cb, scalar1=w_qc)
                nc.vector.tensor_copy(out=cswq[:, N : N + 2], in_=wvec[:, 0:2])
                # qswc = [Q * w_qc | w_c w_c]  -> rhs for X matmuls
                qswc = pool_sc.tile([D, M + 2], F32R, tag=f"qswc{u}")
                nc.vector.tensor_scalar_mul(out=qswc[:, 0:M], in0=qb, scalar1=w_qc)
                nc.vector.tensor_copy(out=qswc[:, M : M + 2], in_=wvec[:, 2:4])

                # --- transposes: ct_k = [Ct_k | 1 1], qtg_j = [Qt_j | 1 1 | G_j]
                ct = pool_ct.tile([128, NK, D + 2], CDT, tag=f"ct{u}")
                for k in range(NK):
                    pt = pp_t.tile([128, 128], F32R, tag="pt")
                    nc.tensor.transpose(pt, cb[:, k * 128 : (k + 1) * 128], ident)
                    nc.vector.tensor_copy(out=ct[:, k, 0:D], in_=pt.bitcast(F32))
                    nc.vector.tensor_copy(out=ct[:, k, D : D + 2], in_=ones2)

                qtg = pool_qtg.tile([128, MJ, 2 * D + 2], F32R, tag=f"qtg{u}")
                for j in range(MJ):
                    pt = pp_t.tile([128, 128], F32R, tag="pt")
                    nc.tensor.transpose(pt, qb[:, j * 128 : (j + 1) * 128], ident)
                    nc.vector.tensor_copy(out=qtg[:, j, 0:D], in_=pt)
                    nc.vector.tensor_copy(out=qtg[:, j, D : D + 2], in_=ones2)

                # --- X [n,m] chunks + col-softmax numerator E ---
                e_col = pool_e.tile([128, NK, M], CDT, tag=f"e_col{u}")
                for k in range(NK):
                    px = pp_x.tile([128, M + 2], F32, tag="px")
                    nc.tensor.matmul(
                        px, cb[:, k * 128 : (k + 1) * 128], qswc, start=True, stop=True
                    )
                    cs_k = pool_sm.tile([128, 1], F32, tag=f"cs{k}_{u}")
                    nc.vector.tensor_copy(out=cs_k, in_=px[:, M : M + 1])
                    nc.scalar.activation(
                        out=e_col[:, k, :],
                        in_=px[:, 0:M],
                        func=mybir.ActivationFunctionType.Exp,
                        bias=cs_k,
                    )

                # --- X^T [m,n] chunks + row-softmax numerator E' ---
                e_row = pool_e.tile([128, MJ, N], F32R, tag=f"e_row{u}")
                for j in range(MJ):
                    qbj = qb[:, j * 128 : (j + 1) * 128]
                    pxt = pp_xt.tile([128, N], F32, tag="pxt")
                    for h in range(N // 512):
                        nc.tensor.matmul(
                            pxt[:, h * 512 : (h + 1) * 512],
                            qbj,
                            cswq[:, h * 512 : (h + 1) * 512],
                            start=True,
                            stop=True,
                        )
                    pq = pp_t.tile([128, 128], F32, tag="pt")
                    nc.tensor.matmul(
                        pq[:, 0:2], qbj, cswq[:, N : N + 2], start=True, stop=True
                    )
                    qs_j = pool_sm.tile([128, 1], F32, tag=f"qs{j}_{u}")
                    nc.vector.tensor_copy(out=qs_j, in_=pq[:, 0:1])
                    nc.scalar.activation(
                        out=e_row[:, j, :],
                        in_=pxt,
                        func=mybir.ActivationFunctionType.Exp,
                        bias=qs_j,
                    )

                # --- col path: G_j = normalize(E^T @ [Ct|1 1]) ---
                for j in range(MJ):
                    pg = pp_t.tile([128, D + 2], F32, tag="pt")
                    for k in range(NK):
                        nc.tensor.matmul(
                            pg,
                            e_col[:, k, j * 128 : (j + 1) * 128],
                            ct[:, k, :],
                            start=(k == 0),
                            stop=(k == NK - 1),
                        )
                    rcol = pool_sm.tile([128, 1], F32, tag=f"rcol{j}_{u}")
                    nc.vector.reciprocal(out=rcol, in_=pg[:, D : D + 1])
                    nc.vector.tensor_scalar_mul(
                        out=qtg[:, j, D + 2 : 2 * D + 2], in0=pg[:, 0:D], scalar1=rcol
                    )

                # --- row path: [A | rowsum rowsum | Bt] = E'^T @ [Qt|1 1|G] ---
                if merged_out:
                    oabs = pool_out.tile([128, NK, 2 * D], ODT, tag=f"oabs{u}")
                for k in range(NK):
                    pab = pp_x.tile([128, 2 * D + 2], F32, tag="px")
                    for j in range(MJ):
                        nc.tensor.matmul(
                            pab,
                            e_row[:, j, k * 128 : (k + 1) * 128],
                            qtg[:, j, :],
                            start=(j == 0),
                            stop=(j == MJ - 1),
                        )
                    rrow = pool_sm.tile([128, 1], F32, tag=f"rrow{k}_{u}")
                    nc.vector.reciprocal(out=rrow, in_=pab[:, D : D + 1])
                    oab = oabs[:, k, :] if merged_out else pool_out.tile(
                        [128, 2 * D], ODT, tag="oab"
                    )
                    nc.vector.tensor_scalar_mul(
                        out=oab[:, 0:D], in0=pab[:, 0:D], scalar1=rrow
                    )
                    nc.vector.tensor_scalar_mul(
                        out=oab[:, D : 2 * D], in0=pab[:, D + 2 : 2 * D + 2], scalar1=rrow
                    )
                    if not merged_out:
                        nc.sync.dma_start(
                            out=Af[ts(b * NK + k, 128), :], in_=oab[:, 0:D]
                        )
                        nc.sync.dma_start(
                            out=Bf[ts(b * NK + k, 128), :], in_=oab[:, D : 2 * D]
                        )
                if merged_out:
                    # one 3D DMA per output per batch: SBUF [p, k, d] -> DRAM
                    # rows (b*N + k*128 + p)
                    nc.sync.dma_start(
                        out=Af[ts(b, N), :].rearrange("(k p) d -> p k d", p=128),
                        in_=oabs[:, :, 0:D],
                    )
                    nc.sync.dma_start(
                        out=Bf[ts(b, N), :].rearrange("(k p) d -> p k d", p=128),
                        in_=oabs[:, :, D : 2 * D],
                    )
    nc.finalize()
    return nc


_NC_CACHE = None


def kernel(C, Q, W0, b0, _trace=False):
    global _NC_CACHE
    if _NC_CACHE is None:
        _NC_CACHE = build_kernel()
    nc = _NC_CACHE

    C = np.ascontiguousarray(np.asarray(C, dtype=np.float32))
    Q = np.ascontiguousarray(np.asarray(Q, dtype=np.float32))
    W0 = np.ascontiguousarray(np.asarray(W0, dtype=np.float32))

    in_maps = [
        {
            "C": C[i * BPC : (i + 1) * BPC],
            "Q": Q[i * BPC : (i + 1) * BPC],
            "W0": W0,
        }
        for i in range(NCORES)
    ]
    res = run_bass_kernel_spmd(nc, in_maps, core_ids=list(range(NCORES)))
    A = np.concatenate(
        [np.asarray(res.results[i]["A"]) for i in range(NCORES)], axis=0
    ).astype(np.float32)
    Bt = np.concatenate(
        [np.asarray(res.results[i]["Bt"]) for i in range(NCORES)], axis=0
    ).astype(np.float32)
    return (A, Bt)


# revision 30
# speedup vs baseline: 1.1943x; 1.0968x over previous
"""Trainium2 Bass kernel for ContextQueryAttention (trilinear attention).

Math (per batch b; C:[D,N], Q:[D,M], W0:[3D], b0:[1]):
    Ct = C.T, Qt = Q.T
    S[n,m] = Ct@w_c [n] + Qt@w_q [m] + sum_d Ct[n,d]*w_qc[d]*Qt[m,d] + b0
    S_row = softmax_m(S), S_col = softmax_n(S)
    A  = S_row @ Qt                       # (N, D)
    Bt = (S_row @ S_col.T) @ Ct           # (N, D)

Key algebraic restructurings used here:
  * Bt = S_row @ (S_col.T @ Ct)  -- drops the N x N intermediate entirely
    (805 MFLOP/batch -> 134 MFLOP/batch).
  * softmax_m is invariant to per-row constants, softmax_n to per-column
    constants, so the row path only needs the q-score bias and the col path
    only the c-score bias; b0 cancels everywhere.
  * Input magnitudes are O(5), so exp() needs no max-subtraction.
  * Softmax denominators come for free as extra all-ones columns fused
    into the consuming matmuls; normalization folds into per-partition
    scalar multiplies after the matmuls.
  * All matmuls run in float32r (full-rate fp32); moving free sizes kept
    even (hw requirement) by duplicating the fused score/ones columns.

Distribution: ALL 64 batches on ONE core, looped with a hardware For_i
(unroll=4, staggered_reset back-edges, merged 3D output DMAs).
Rationale (measured on the axon-tunneled PJRT path):
  * Per-call dispatch overhead dominates the amortized exec time and
    scales with the number of devices (8-core ~12ms, 2-core ~8ms,
    1-core ~2.4-3ms per call) while the device compute (<1ms) hides
    behind the dispatch pipeline. One core minimizes the metric; the
    sharding_hint's 8-way data parallelism loses 4-5x here.
  * Steady-state per-call cost also grows with NEFF size (instruction
    stream + DMA descriptors): fully unrolling 64 batches costs ~1.5ms
    extra per call. For_i with a 4-batch unrolled body keeps the stream
    ~700 instructions; staggered_reset avoids the ~2us full-barrier
    back-edge and restores cross-batch engine overlap; one 3D DMA per
    output per batch replaces 16 chunk DMAs.
"""

import numpy as np

import concourse.bass as bass
import concourse.bacc as bacc
import concourse.tile as tile
from concourse import mybir
from concourse.bass import ts
from concourse.bass_utils import run_bass_kernel_spmd
from concourse.masks import make_identity

F32 = mybir.dt.float32
F32R = mybir.dt.float32r
BF16 = mybir.dt.bfloat16

# Problem shape (hardcoded per spec)
B, D, N, M = 64, 128, 1024, 256
NCORES = 1
BPC = B // NCORES  # batches per core
NK = N // 128      # context chunks (8)
MJ = M // 128      # query chunks (2)


def build_kernel(
    bpc: int = BPC,
    dynamic: bool = True,
    unroll: int = 4,
    merged_out: bool = True,
    staggered: bool = True,
    out_bf16: bool = False,
    col_bf16: bool = False,
) -> bass.Bass:
    ODT = BF16 if out_bf16 else F32
    # col-path operand dtype: bf16 lifts the fp32r small-moving-dim (130 < 256)
    # 4-cycles/row matmul penalty to full rate, with no extra conversions
    # (e_col is an ACT output, ct a PSUM->SBUF copy; both cast on write).
    CDT = BF16 if col_bf16 else F32R
    nc = bacc.Bacc("TRN2", target_bir_lowering=False, debug=False)

    C8 = nc.dram_tensor("C", [bpc, D, N], F32, kind="ExternalInput").ap()
    Q8 = nc.dram_tensor("Q", [bpc, D, M], F32, kind="ExternalInput").ap()
    W0 = nc.dram_tensor("W0", [3 * D], F32, kind="ExternalInput").ap()
    A8 = nc.dram_tensor("A", [bpc, N, D], ODT, kind="ExternalOutput").ap()
    B8 = nc.dram_tensor("Bt", [bpc, N, D], ODT, kind="ExternalOutput").ap()

    # flat row views for dynamic (runtime-index) batch addressing
    Cf = C8.rearrange("b d n -> (b d) n")
    Qf = Q8.rearrange("b d n -> (b d) n")
    Af = A8.rearrange("b n d -> (b n) d")
    Bf = B8.rearrange("b n d -> (b n) d")

    with tile.TileContext(nc) as tc:
        with (
            tc.tile_pool(name="singles", bufs=1) as singles,
            tc.tile_pool(name="inp", bufs=2) as pool_in,
            tc.tile_pool(name="scaled", bufs=2) as pool_sc,
            tc.tile_pool(name="ct", bufs=2) as pool_ct,
            tc.tile_pool(name="e", bufs=2) as pool_e,
            tc.tile_pool(name="qtg", bufs=2) as pool_qtg,
            tc.tile_pool(name="small", bufs=2) as pool_sm,
            tc.tile_pool(name="out", bufs=2) as pool_out,
            tc.tile_pool(name="pp_t", bufs=2, space="PSUM") as pp_t,
            tc.tile_pool(name="pp_x", bufs=2, space="PSUM") as pp_x,
            tc.tile_pool(name="pp_xt", bufs=2, space="PSUM") as pp_xt,
        ):
            # --- constants ---
            # wvec cols: w_q, w_q, w_c, w_c, w_qc  (score columns doubled so
            # fused matmul moving sizes stay even, as float32r requires)
            wvec = singles.tile([D, 5], F32)
            for i, s in enumerate((0, 0, 1, 1, 2)):
                nc.sync.dma_start(
                    out=wvec[:, i : i + 1],
                    in_=W0[s * D : (s + 1) * D].rearrange("(p o) -> p o", o=1),
                )
            w_qc = wvec[:, 4:5]
            ones2 = singles.tile([128, 2], F32)
            nc.vector.memset(ones2, 1.0)
            ident_f32 = singles.tile([128, 128], F32)
            make_identity(nc, ident_f32)
            ident = singles.tile([128, 128], F32R)
            nc.vector.tensor_copy(out=ident, in_=ident_f32)
            wq2 = singles.tile([D, 2], F32R)
            nc.vector.tensor_copy(out=wq2, in_=wvec[:, 0:2])

            import contextlib

            loop_ctx = (
                tc.For_i(0, bpc, unroll, staggered_reset=staggered)
                if dynamic
                else contextlib.nullcontext(0)
            )
            with loop_ctx as bdyn:
              for bstat in range(unroll if dynamic else bpc):
                b = (bdyn + bstat) if dynamic else bstat
                u = bstat % 2  # alternate tags so consecutive batches overlap
                cb = pool_in.tile([D, N], F32R, tag=f"cb{u}")
                qb = pool_in.tile([D, M], F32R, tag=f"qb{u}")
                nc.sync.dma_start(out=cb, in_=Cf[ts(b, D), :].bitcast(F32R))
                nc.sync.dma_start(out=qb, in_=Qf[ts(b, D), :].bitcast(F32R))

                # qswc = [Q * w_qc | w_c w_c]  -> rhs for X matmuls; its scaled-Q
                # columns double as the stationary operand for the X^T matmuls
                # (sum_d (Q w_qc)[d,m] C[d,n] needs only ONE scaled tensor, so
                # no N-wide scaled copy of C is ever built).
                qswc = pool_sc.tile([D, M + 2], F32R, tag=f"qswc{u}")
                nc.vector.tensor_scalar_mul(out=qswc[:, 0:M], in0=qb, scalar1=w_qc)
                nc.vector.tensor_copy(out=qswc[:, M : M + 2], in_=wvec[:, 2:4])

                # --- transposes: ct_k = [Ct_k | 1 1], qtg_j = [Qt_j | 1 1 | G_j]
                ct = pool_ct.tile([128, NK, D + 2], CDT, tag=f"ct{u}")
                for k in range(NK):
                    pt = pp_t.tile([128, 128], F32R, tag="pt")
                    nc.tensor.transpose(pt, cb[:, k * 128 : (k + 1) * 128], ident)
                    nc.vector.tensor_copy(out=ct[:, k, 0:D], in_=pt.bitcast(F32))
                    nc.vector.tensor_copy(out=ct[:, k, D : D + 2], in_=ones2)

                qtg = pool_qtg.tile([128, MJ, 2 * D + 2], F32R, tag=f"qtg{u}")
                for j in range(MJ):
                    pt = pp_t.tile([128, 128], F32R, tag="pt")
                    nc.tensor.transpose(pt, qb[:, j * 128 : (j + 1) * 128], ident)
                    nc.vector.tensor_copy(out=qtg[:, j, 0:D], in_=pt)
                    nc.vector.tensor_copy(out=qtg[:, j, D : D + 2], in_=ones2)

                # --- X [n,m] chunks + col-softmax numerator E ---
                e_col = pool_e.tile([128, NK, M], CDT, tag=f"e_col{u}")
                for k in range(NK):
                    px = pp_x.tile([128, M + 2], F32, tag="px")
                    nc.tensor.matmul(
                        px, cb[:, k * 128 : (k + 1) * 128], qswc, start=True, stop=True
                    )
                    cs_k = pool_sm.tile([128, 1], F32, tag=f"cs{k}_{u}")
                    nc.vector.tensor_copy(out=cs_k, in_=px[:, M : M + 1])
                    nc.scalar.activation(
                        out=e_col[:, k, :],
                        in_=px[:, 0:M],
                        func=mybir.ActivationFunctionType.Exp,
                        bias=cs_k,
                    )

                # --- X^T [m,n] chunks + row-softmax numerator E' ---
                e_row = pool_e.tile([128, MJ, N], F32R, tag=f"e_row{u}")
                for j in range(MJ):
                    qbj = qb[:, j * 128 : (j + 1) * 128]
                    qsj = qswc[:, j * 128 : (j + 1) * 128]  # (Q*w_qc) chunk
                    pxt = pp_xt.tile([128, N], F32, tag="pxt")
                    for h in range(N // 512):
                        nc.tensor.matmul(
                            pxt[:, h * 512 : (h + 1) * 512],
                            qsj,
                            cb[:, h * 512 : (h + 1) * 512],
                            start=True,
                            stop=True,
                        )
                    pq = pp_t.tile([128, 128], F32, tag="pt")
                    nc.tensor.matmul(
                        pq[:, 0:2], qbj, wq2, start=True, stop=True
                    )
                    qs_j = pool_sm.tile([128, 1], F32, tag=f"qs{j}_{u}")
                    nc.vector.tensor_copy(out=qs_j, in_=pq[:, 0:1])
                    nc.scalar.activation(
                        out=e_row[:, j, :],
                        in_=pxt,
                        func=mybir.ActivationFunctionType.Exp,
                        bias=qs_j,
                    )

                # --- col path: G_j = normalize(E^T @ [Ct|1 1]) ---
                for j in range(MJ):
                    pg = pp_t.tile([128, D + 2], F32, tag="pt")
                    for k in range(NK):
                        nc.tensor.matmul(
                            pg,
                            e_col[:, k, j * 128 : (j + 1) * 128],
                            ct[:, k, :],
                            start=(k == 0),
                            stop=(k == NK - 1),
                        )
                    rcol = pool_sm.tile([128, 1], F32, tag=f"rcol{j}_{u}")
                    nc.vector.reciprocal(out=rcol, in_=pg[:, D : D + 1])
                    nc.vector.tensor_scalar_mul(
                        out=qtg[:, j, D + 2 : 2 * D + 2], in0=pg[:, 0:D], scalar1=rcol
                    )

                # --- row path: [A | rowsum rowsum | Bt] = E'^T @ [Qt|1 1|G] ---
                if merged_out:
                    oabs = pool_out.tile([128, NK, 2 * D], ODT, tag=f"oabs{u}")
                for k in range(NK):
                    pab = pp_x.tile([128, 2 * D + 2], F32, tag="px")
                    for j in range(MJ):
                        nc.tensor.matmul(
                            pab,
                            e_row[:, j, k * 128 : (k + 1) * 128],
                            qtg[:, j, :],
                            start=(j == 0),
                            stop=(j == MJ - 1),
                        )
                    rrow = pool_sm.tile([128, 1], F32, tag=f"rrow{k}_{u}")
                    nc.vector.reciprocal(out=rrow, in_=pab[:, D : D + 1])
                    oab = oabs[:, k, :] if merged_out else pool_out.tile(
                        [128, 2 * D], ODT, tag="oab"
                    )
                    nc.vector.tensor_scalar_mul(
                        out=oab[:, 0:D], in0=pab[:, 0:D], scalar1=rrow
                    )
                    nc.vector.tensor_scalar_mul(
                        out=oab[:, D : 2 * D], in0=pab[:, D + 2 : 2 * D + 2], scalar1=rrow
                    )
                    if not merged_out:
                        nc.sync.dma_start(
                            out=Af[ts(b * NK + k, 128), :], in_=oab[:, 0:D]
                        )
                        nc.sync.dma_start(
                            out=Bf[ts(b * NK + k, 128), :], in_=oab[:, D : 2 * D]
                        )
                if merged_out:
                    # one 3D DMA per output per batch: SBUF [p, k, d] -> DRAM
                    # rows (b*N + k*128 + p)
                    nc.sync.dma_start(
                        out=Af[ts(b, N), :].rearrange("(k p) d -> p k d", p=128),
                        in_=oabs[:, :, 0:D],
                    )
                    nc.sync.dma_start(
                        out=Bf[ts(b, N), :].rearrange("(k p) d -> p k d", p=128),
                        in_=oabs[:, :, D : 2 * D],
                    )
    nc.finalize()
    return nc


_NC_CACHE = None


def kernel(C, Q, W0, b0, _trace=False):
    global _NC_CACHE
    if _NC_CACHE is None:
        _NC_CACHE = build_kernel()
    nc = _NC_CACHE

    C = np.ascontiguousarray(np.asarray(C, dtype=np.float32))
    Q = np.ascontiguousarray(np.asarray(Q, dtype=np.float32))
    W0 = np.ascontiguousarray(np.asarray(W0, dtype=np.float32))

    in_maps = [
        {
            "C": C[i * BPC : (i + 1) * BPC],
            "Q": Q[i * BPC : (i + 1) * BPC],
            "W0": W0,
        }
        for i in range(NCORES)
    ]
    res = run_bass_kernel_spmd(nc, in_maps, core_ids=list(range(NCORES)))
    A = np.concatenate(
        [np.asarray(res.results[i]["A"]) for i in range(NCORES)], axis=0
    ).astype(np.float32)
    Bt = np.concatenate(
        [np.asarray(res.results[i]["Bt"]) for i in range(NCORES)], axis=0
    ).astype(np.float32)
    return (A, Bt)


# revision 31
# speedup vs baseline: 1.5384x; 1.2881x over previous
"""Trainium2 Bass kernel for ContextQueryAttention (trilinear attention).

Math (per batch b; C:[D,N], Q:[D,M], W0:[3D], b0:[1]):
    Ct = C.T, Qt = Q.T
    S[n,m] = Ct@w_c [n] + Qt@w_q [m] + sum_d Ct[n,d]*w_qc[d]*Qt[m,d] + b0
    S_row = softmax_m(S), S_col = softmax_n(S)
    A  = S_row @ Qt                       # (N, D)
    Bt = (S_row @ S_col.T) @ Ct           # (N, D)

Key algebraic restructurings used here:
  * Bt = S_row @ (S_col.T @ Ct)  -- drops the N x N intermediate entirely
    (805 MFLOP/batch -> 134 MFLOP/batch).
  * softmax_m is invariant to per-row constants, softmax_n to per-column
    constants, so the row path only needs the q-score bias and the col path
    only the c-score bias; b0 cancels everywhere.
  * Input magnitudes are O(5), so exp() needs no max-subtraction.
  * Softmax denominators come for free as extra all-ones columns fused
    into the consuming matmuls; normalization folds into per-partition
    scalar multiplies after the matmuls.
  * All matmuls run in float32r (full-rate fp32); moving free sizes kept
    even (hw requirement) by duplicating the fused score/ones columns.

Distribution: ALL 64 batches on ONE core, looped with a hardware For_i
(unroll=4, staggered_reset back-edges, merged 3D output DMAs).
Rationale (measured on the axon-tunneled PJRT path):
  * Per-call dispatch overhead dominates the amortized exec time and
    scales with the number of devices (8-core ~12ms, 2-core ~8ms,
    1-core ~2.4-3ms per call) while the device compute (<1ms) hides
    behind the dispatch pipeline. One core minimizes the metric; the
    sharding_hint's 8-way data parallelism loses 4-5x here.
  * Steady-state per-call cost also grows with NEFF size (instruction
    stream + DMA descriptors): fully unrolling 64 batches costs ~1.5ms
    extra per call. For_i with a 4-batch unrolled body keeps the stream
    ~700 instructions; staggered_reset avoids the ~2us full-barrier
    back-edge and restores cross-batch engine overlap; one 3D DMA per
    output per batch replaces 16 chunk DMAs.
"""

import numpy as np

import concourse.bass as bass
import concourse.bacc as bacc
import concourse.tile as tile
from concourse import mybir
from concourse.bass import ts
from concourse.bass_utils import run_bass_kernel_spmd
from concourse.masks import make_identity

F32 = mybir.dt.float32
F32R = mybir.dt.float32r
BF16 = mybir.dt.bfloat16

# Problem shape (hardcoded per spec)
B, D, N, M = 64, 128, 1024, 256
NCORES = 1
BPC = B // NCORES  # batches per core
NK = N // 128      # context chunks (8)
MJ = M // 128      # query chunks (2)


def build_kernel(
    bpc: int = BPC,
    dynamic: bool = True,
    unroll: int = 4,
    merged_out: bool = True,
    staggered: bool = True,
    out_bf16: bool = False,
    col_bf16: bool = True,
) -> bass.Bass:
    ODT = BF16 if out_bf16 else F32
    # col-path operand dtype: bf16 lifts the fp32r small-moving-dim (130 < 256)
    # 4-cycles/row matmul penalty to full rate, with no extra conversions
    # (e_col is an ACT output, ct a PSUM->SBUF copy; both cast on write).
    CDT = BF16 if col_bf16 else F32R
    nc = bacc.Bacc("TRN2", target_bir_lowering=False, debug=False)

    C8 = nc.dram_tensor("C", [bpc, D, N], F32, kind="ExternalInput").ap()
    Q8 = nc.dram_tensor("Q", [bpc, D, M], F32, kind="ExternalInput").ap()
    W0 = nc.dram_tensor("W0", [3 * D], F32, kind="ExternalInput").ap()
    A8 = nc.dram_tensor("A", [bpc, N, D], ODT, kind="ExternalOutput").ap()
    B8 = nc.dram_tensor("Bt", [bpc, N, D], ODT, kind="ExternalOutput").ap()

    # flat row views for dynamic (runtime-index) batch addressing
    Cf = C8.rearrange("b d n -> (b d) n")
    Qf = Q8.rearrange("b d n -> (b d) n")
    Af = A8.rearrange("b n d -> (b n) d")
    Bf = B8.rearrange("b n d -> (b n) d")

    with tile.TileContext(nc) as tc:
        with (
            tc.tile_pool(name="singles", bufs=1) as singles,
            tc.tile_pool(name="inp", bufs=2) as pool_in,
            tc.tile_pool(name="scaled", bufs=2) as pool_sc,
            tc.tile_pool(name="ct", bufs=2) as pool_ct,
            tc.tile_pool(name="e", bufs=2) as pool_e,
            tc.tile_pool(name="qtg", bufs=2) as pool_qtg,
            tc.tile_pool(name="small", bufs=2) as pool_sm,
            tc.tile_pool(name="out", bufs=2) as pool_out,
            tc.tile_pool(name="pp_t", bufs=2, space="PSUM") as pp_t,
            tc.tile_pool(name="pp_x", bufs=2, space="PSUM") as pp_x,
            tc.tile_pool(name="pp_xt", bufs=2, space="PSUM") as pp_xt,
        ):
            # --- constants ---
            # wvec cols: w_q, w_q, w_c, w_c, w_qc  (score columns doubled so
            # fused matmul moving sizes stay even, as float32r requires)
            wvec = singles.tile([D, 5], F32)
            for i, s in enumerate((0, 0, 1, 1, 2)):
                nc.sync.dma_start(
                    out=wvec[:, i : i + 1],
                    in_=W0[s * D : (s + 1) * D].rearrange("(p o) -> p o", o=1),
                )
            w_qc = wvec[:, 4:5]
            ones2 = singles.tile([128, 2], F32)
            nc.vector.memset(ones2, 1.0)
            ident_f32 = singles.tile([128, 128], F32)
            make_identity(nc, ident_f32)
            ident = singles.tile([128, 128], F32R)
            nc.vector.tensor_copy(out=ident, in_=ident_f32)
            wq2 = singles.tile([D, 2], F32R)
            nc.vector.tensor_copy(out=wq2, in_=wvec[:, 0:2])

            import contextlib

            loop_ctx = (
                tc.For_i(0, bpc, unroll, staggered_reset=staggered)
                if dynamic
                else contextlib.nullcontext(0)
            )
            with loop_ctx as bdyn:
              for bstat in range(unroll if dynamic else bpc):
                b = (bdyn + bstat) if dynamic else bstat
                u = bstat % 2  # alternate tags so consecutive batches overlap
                cb = pool_in.tile([D, N], F32R, tag=f"cb{u}")
                qb = pool_in.tile([D, M], F32R, tag=f"qb{u}")
                nc.sync.dma_start(out=cb, in_=Cf[ts(b, D), :].bitcast(F32R))
                nc.sync.dma_start(out=qb, in_=Qf[ts(b, D), :].bitcast(F32R))

                # qswc = [Q * w_qc | w_c w_c]  -> rhs for X matmuls; its scaled-Q
                # columns double as the stationary operand for the X^T matmuls
                # (sum_d (Q w_qc)[d,m] C[d,n] needs only ONE scaled tensor, so
                # no N-wide scaled copy of C is ever built).
                qswc = pool_sc.tile([D, M + 2], F32R, tag=f"qswc{u}")
                nc.vector.tensor_scalar_mul(out=qswc[:, 0:M], in0=qb, scalar1=w_qc)
                nc.vector.tensor_copy(out=qswc[:, M : M + 2], in_=wvec[:, 2:4])

                # --- transposes: ct_k = [Ct_k | 1 1], qtg_j = [Qt_j | 1 1 | G_j]
                ct = pool_ct.tile([128, NK, D + 2], CDT, tag=f"ct{u}")
                for k in range(NK):
                    pt = pp_t.tile([128, 128], F32R, tag="pt")
                    nc.tensor.transpose(pt, cb[:, k * 128 : (k + 1) * 128], ident)
                    nc.vector.tensor_copy(out=ct[:, k, 0:D], in_=pt.bitcast(F32))
                    nc.vector.tensor_copy(out=ct[:, k, D : D + 2], in_=ones2)

                qtg = pool_qtg.tile([128, MJ, 2 * D + 2], F32R, tag=f"qtg{u}")
                for j in range(MJ):
                    pt = pp_t.tile([128, 128], F32R, tag="pt")
                    nc.tensor.transpose(pt, qb[:, j * 128 : (j + 1) * 128], ident)
                    nc.vector.tensor_copy(out=qtg[:, j, 0:D], in_=pt)
                    nc.vector.tensor_copy(out=qtg[:, j, D : D + 2], in_=ones2)

                # --- X [n,m] chunks + col-softmax numerator E ---
                e_col = pool_e.tile([128, NK, M], CDT, tag=f"e_col{u}")
                for k in range(NK):
                    px = pp_x.tile([128, M + 2], F32, tag="px")
                    nc.tensor.matmul(
                        px, cb[:, k * 128 : (k + 1) * 128], qswc, start=True, stop=True
                    )
                    cs_k = pool_sm.tile([128, 1], F32, tag=f"cs{k}_{u}")
                    nc.vector.tensor_copy(out=cs_k, in_=px[:, M : M + 1])
                    nc.scalar.activation(
                        out=e_col[:, k, :],
                        in_=px[:, 0:M],
                        func=mybir.ActivationFunctionType.Exp,
                        bias=cs_k,
                    )

                # --- X^T [m,n] chunks + row-softmax numerator E' ---
                e_row = pool_e.tile([128, MJ, N], F32R, tag=f"e_row{u}")
                for j in range(MJ):
                    qbj = qb[:, j * 128 : (j + 1) * 128]
                    qsj = qswc[:, j * 128 : (j + 1) * 128]  # (Q*w_qc) chunk
                    pxt = pp_xt.tile([128, N], F32, tag="pxt")
                    for h in range(N // 512):
                        nc.tensor.matmul(
                            pxt[:, h * 512 : (h + 1) * 512],
                            qsj,
                            cb[:, h * 512 : (h + 1) * 512],
                            start=True,
                            stop=True,
                        )
                    pq = pp_t.tile([128, 128], F32, tag="pt")
                    nc.tensor.matmul(
                        pq[:, 0:2], qbj, wq2, start=True, stop=True
                    )
                    qs_j = pool_sm.tile([128, 1], F32, tag=f"qs{j}_{u}")
                    nc.vector.tensor_copy(out=qs_j, in_=pq[:, 0:1])
                    nc.scalar.activation(
                        out=e_row[:, j, :],
                        in_=pxt,
                        func=mybir.ActivationFunctionType.Exp,
                        bias=qs_j,
                    )

                # --- col path: G_j = normalize(E^T @ [Ct|1 1]) ---
                for j in range(MJ):
                    pg = pp_t.tile([128, D + 2], F32, tag="pt")
                    for k in range(NK):
                        nc.tensor.matmul(
                            pg,
                            e_col[:, k, j * 128 : (j + 1) * 128],
                            ct[:, k, :],
                            start=(k == 0),
                            stop=(k == NK - 1),
                        )
                    rcol = pool_sm.tile([128, 1], F32, tag=f"rcol{j}_{u}")
                    nc.vector.reciprocal(out=rcol, in_=pg[:, D : D + 1])
                    nc.vector.tensor_scalar_mul(
                        out=qtg[:, j, D + 2 : 2 * D + 2], in0=pg[:, 0:D], scalar1=rcol
                    )

                # --- row path: [A | rowsum rowsum | Bt] = E'^T @ [Qt|1 1|G] ---
                if merged_out:
                    oabs = pool_out.tile([128, NK, 2 * D], ODT, tag=f"oabs{u}")
                for k in range(NK):
                    pab = pp_x.tile([128, 2 * D + 2], F32, tag="px")
                    for j in range(MJ):
                        nc.tensor.matmul(
                            pab,
                            e_row[:, j, k * 128 : (k + 1) * 128],
                            qtg[:, j, :],
                            start=(j == 0),
                            stop=(j == MJ - 1),
                        )
                    rrow = pool_sm.tile([128, 1], F32, tag=f"rrow{k}_{u}")
                    nc.vector.reciprocal(out=rrow, in_=pab[:, D : D + 1])
                    oab = oabs[:, k, :] if merged_out else pool_out.tile(
                        [128, 2 * D], ODT, tag="oab"
                    )
                    nc.vector.tensor_scalar_mul(
                        out=oab[:, 0:D], in0=pab[:, 0:D], scalar1=rrow
                    )
                    nc.vector.tensor_scalar_mul(
                        out=oab[:, D : 2 * D], in0=pab[:, D + 2 : 2 * D + 2], scalar1=rrow
                    )
                    if not merged_out:
                        nc.sync.dma_start(
                            out=Af[ts(b * NK + k, 128), :], in_=oab[:, 0:D]
                        )
                        nc.sync.dma_start(
                            out=Bf[ts(b * NK + k, 128), :], in_=oab[:, D : 2 * D]
                        )
                if merged_out:
                    # one 3D DMA per output per batch: SBUF [p, k, d] -> DRAM
                    # rows (b*N + k*128 + p)
                    nc.sync.dma_start(
                        out=Af[ts(b, N), :].rearrange("(k p) d -> p k d", p=128),
                        in_=oabs[:, :, 0:D],
                    )
                    nc.sync.dma_start(
                        out=Bf[ts(b, N), :].rearrange("(k p) d -> p k d", p=128),
                        in_=oabs[:, :, D : 2 * D],
                    )
    nc.finalize()
    return nc


_NC_CACHE = None


def kernel(C, Q, W0, b0, _trace=False):
    global _NC_CACHE
    if _NC_CACHE is None:
        _NC_CACHE = build_kernel()
    nc = _NC_CACHE

    C = np.ascontiguousarray(np.asarray(C, dtype=np.float32))
    Q = np.ascontiguousarray(np.asarray(Q, dtype=np.float32))
    W0 = np.ascontiguousarray(np.asarray(W0, dtype=np.float32))

    in_maps = [
        {
            "C": C[i * BPC : (i + 1) * BPC],
            "Q": Q[i * BPC : (i + 1) * BPC],
            "W0": W0,
        }
        for i in range(NCORES)
    ]
    res = run_bass_kernel_spmd(nc, in_maps, core_ids=list(range(NCORES)))
    A = np.concatenate(
        [np.asarray(res.results[i]["A"]) for i in range(NCORES)], axis=0
    ).astype(np.float32)
    Bt = np.concatenate(
        [np.asarray(res.results[i]["Bt"]) for i in range(NCORES)], axis=0
    ).astype(np.float32)
    return (A, Bt)
